# revision 61
# baseline (speedup 1.0000x reference)
"""Trainium2 Bass kernel for nn_MultiHeadAttention_7413113553038.

Sharding: 8 cores = (batch b in {0,1}) x (query block of 512). Each core
computes all 4 heads of attention for its 512 queries against the full 2048
keys of its batch, plus the output projection, residual add and LayerNorm
for its rows. No collectives needed.

Per-core strategy:
  - Host pre-packs X_Q^T (block), X_K^T/X_V^T (transposed + rolled so the
    causal Gaussian band sits at k-chunks 10-15), fp16 weights and the
    analytic band tables into two HWDGE queues of need-ordered descriptors.
  - PE warm-up dummy matmuls bridge the DMA window so the HAM clock gate
    (PE at 1.2 GHz until ~3.4us of sustained activity) opens before real
    work starts.
  - Q^T/K^T computed in [d, seq] layout (lhsT = W, rhs = X^T); V per
    128-key chunk with a ones-column so each PV matmul also accumulates
    the softmax denominator Z as psum row 64.
  - Attention runs head-PAIRS (scores psum [128, 2 heads, 512 q] per
    chunk, contraction d=64 at partition bases 0/64). One 1024-wide exp
    per chunk covers both heads, alternating engines per chunk: even
    chunks use the scalar activation, odd chunks a Schraudolph bit-trick
    on the DVE (s*1477.32+15316 -> int16 -> bitcast fp16, ~3% sawtooth
    that cancels through p/Z). Gaussian band bias: DVE chunks get a fused
    exp(s+g) overwrite via the GS table (scalar_tensor_tensor); scalar
    chunks get an E=exp(g) multiply on the otherwise idle gpsimd engine.
  - PV matmuls lag 3 chunks behind the scores so exp jitter never stalls
    the PE (HAM stays open); projections of later K/V blocks and the
    per-head 1/Z dances + fc/LayerNorm drip into the chunk streams.
  - 1/Z per head: z row -> PE transposes -> reciprocal [128,4] ->
    transpose back -> selector-matmul broadcast -> ctx scale; group 0's
    dances + fc drip into group 1's stream, only heads 2,3 dance in the
    tail. Output stored fp16 (upcast on host).
"""

import numpy as np

N_HEADS = 4
D_K = 64
B = 2
S = 2048
F = 256
QB = 512  # queries per core
P = 128
KC = S // P  # 16 k-chunks
SIGMA_HS = (5.0, 10.0, 20.0, 40.0)
LN_EPS = 1e-5
N_CORES = 8
# per-head causal-bias band width (g >= ~1e-4): ceil(4.292 * sigma)
BAND = (22, 43, 86, 172)
E01_W = 192
E25_W = 304
N_WARM = 5  # PE warm-up matmuls issued while input DMAs are in flight
# Schraudolph bit-trick exp into fp16 via int16: bits = x*(1024/ln2) +
# 15360 - 44 (centering). Scores are bounded |s+g| <= 9.6 so the int16
# value stays in (0, 32767): no saturation hazards.
K_EXP = 1024.0 / np.log(2.0)
B_EXP = 15360.0 - 44.0


_CACHE = {}


def _gauss_tables():
    """Band tables for the fused bit-trick exp, fp16, transposed-score
    layout (delta = q - k = off_t + j - i, off_t = 256-128t, t = kc-10).

      gs01 [4,128,192]: gs01[h,i,m] = g_h(m - i + 128)*K_EXP + B_EXP
                        (slots t=0,1; slice col = (128 - 128t) + j)
      gs25 [4,128,304]: gs25[h,i,m] = g_h(m - i)*K_EXP + B_EXP
                        (slots t=2..5; slice col = j - 128*(t-2))
    g_h(d) = exp(-d^2 / (2 sigma_h^2)) for d >= 0 else 0.
    """
    i = np.arange(P, dtype=np.float64)[None, :, None]
    sig = np.asarray(SIGMA_HS, dtype=np.float64)[:, None, None]

    m01 = np.arange(E01_W, dtype=np.float64)[None, None, :]
    d01 = m01 - i + 128.0
    g01 = np.where(d01 >= 0, np.exp(-(d01 ** 2) / (2 * sig ** 2)), 0.0)

    m25 = np.arange(E25_W, dtype=np.float64)[None, None, :]
    d25 = m25 - i
    g25 = np.where(d25 >= 0, np.exp(-(d25 ** 2) / (2 * sig ** 2)), 0.0)
    return (
        (g01 * K_EXP + B_EXP).astype(np.float16),
        (g25 * K_EXP + B_EXP).astype(np.float16),
        np.exp(g01).astype(np.float16),
        np.exp(g25).astype(np.float16),
    )


def _build_program():
    import concourse.bass as bass  # noqa: F401
    import concourse.tile as tile
    from concourse import bacc, mybir
    from concourse.masks import make_identity

    f32 = mybir.dt.float32
    f16 = mybir.dt.float16
    f8 = mybir.dt.float8e4
    i16 = mybir.dt.int16
    AF = mybir.ActivationFunctionType
    ALU = mybir.AluOpType
    DR = mybir.MatmulPerfMode.DoubleRow

    nc = bacc.Bacc("TRN2", target_bir_lowering=False, debug=False)

    # inputs pre-packed into per-queue descriptors in first-use order:
    #   sync   [xqt+wq] [wk+kt0] [kt1] [kt2] [kt3] [wfc]
    #   scalar [wv+xvt0] [xvt1] [xvt2] [xvt3] [gs01+gs25] [res]
    s1_d = nc.dram_tensor("s1", [P, 1536], f16, kind="ExternalInput").ap()
    s2_d = nc.dram_tensor("s2", [P, 1536], f16, kind="ExternalInput").ap()
    s3_d = nc.dram_tensor("s3", [P, 1024], f16, kind="ExternalInput").ap()
    s4_d = nc.dram_tensor("s4", [P, 1024], f16, kind="ExternalInput").ap()
    s5_d = nc.dram_tensor("s5", [P, 1024], f16, kind="ExternalInput").ap()
    s6_d = nc.dram_tensor("s6", [P, 512], f16, kind="ExternalInput").ap()
    a1_d = nc.dram_tensor("a1", [P, 1536], f16, kind="ExternalInput").ap()
    a2_d = nc.dram_tensor("a2", [P, 1024], f16, kind="ExternalInput").ap()
    a3_d = nc.dram_tensor("a3", [P, 1024], f16, kind="ExternalInput").ap()
    a4_d = nc.dram_tensor("a4", [P, 1024], f16, kind="ExternalInput").ap()
    a5_d = nc.dram_tensor("a5", [P, 3968], f16, kind="ExternalInput").ap()
    a6_d = nc.dram_tensor("a6", [P, 1024], f16, kind="ExternalInput").ap()
    out = nc.dram_tensor("out", [P, 4, F], f16, kind="ExternalOutput").ap()

    with tile.TileContext(nc) as tc:
        with (
            tc.tile_pool(name="wpool", bufs=1) as wpool,
            tc.tile_pool(name="xpool", bufs=1) as xpool,
            tc.tile_pool(name="proj", bufs=1) as proj,
            tc.tile_pool(name="mmps", bufs=2, space="PSUM") as mmps,
            tc.tile_pool(name="spsum", bufs=2, space="PSUM") as spsum,
            tc.tile_pool(name="cpsum", bufs=2, space="PSUM") as cpsum,
            tc.tile_pool(name="ptpool", bufs=6) as ptpool,
            tc.tile_pool(name="opool", bufs=4) as opool,
        ):
            s1_t = xpool.tile([P, 1536], f16, tag="s1")
            nc.sync.dma_start(s1_t, s1_d)
            a1_t = xpool.tile([P, 1536], f16, tag="a1")
            nc.scalar.dma_start(a1_t, a1_d)
            s2_t = xpool.tile([P, 1536], f16, tag="s2")
            nc.sync.dma_start(s2_t, s2_d)
            a2_t = xpool.tile([P, 1024], f16, tag="a2")
            nc.scalar.dma_start(a2_t, a2_d)
            s3_t = xpool.tile([P, 1024], f16, tag="s3")
            nc.sync.dma_start(s3_t, s3_d)
            a3_t = xpool.tile([P, 1024], f16, tag="a3")
            nc.scalar.dma_start(a3_t, a3_d)
            s4_t = xpool.tile([P, 1024], f16, tag="s4")
            nc.sync.dma_start(s4_t, s4_d)
            a4_t = xpool.tile([P, 1024], f16, tag="a4")
            nc.scalar.dma_start(a4_t, a4_d)
            s5_t = xpool.tile([P, 1024], f16, tag="s5")
            nc.sync.dma_start(s5_t, s5_d)
            a5_t = wpool.tile([P, 3968], f16, tag="a5")
            nc.scalar.dma_start(a5_t, a5_d)
            s6_t = wpool.tile([P, 512], f16, tag="s6")
            nc.sync.dma_start(s6_t, s6_d)
            a6_t = wpool.tile([P, 1024], f16, tag="a6")
            nc.scalar.dma_start(a6_t, a6_d)

            r2 = lambda ap, w: ap.rearrange("p (c f) -> p c f", c=2, f=w)
            r4 = lambda ap, w: ap.rearrange("p (c f) -> p c f", c=4, f=w)
            xqt_sb = r2(s1_t[:, 0:1024], 512)
            wq_sb = r2(s1_t[:, 1024:1536], F)
            wk_sb = r2(s2_t[:, 0:512], F)
            xkt_b = [
                r2(s2_t[:, 512:1536], 512),
                r2(s3_t, 512),
                r2(s4_t, 512),
                r2(s5_t, 512),
            ]
            wfc_sb = r2(s6_t, F)
            wv_sb = r2(a1_t[:, 0:512], F)
            xvt_b = [
                r2(a1_t[:, 512:1536], 512),
                r2(a2_t, 512),
                r2(a3_t, 512),
                r2(a4_t, 512),
            ]
            gs01_sb = r4(a5_t[:, 0:768], E01_W)
            gs25_sb = r4(a5_t[:, 768:1984], E25_W)
            e01_sb = r4(a5_t[:, 1984:2752], E01_W)
            e25_sb = r4(a5_t[:, 2752:3968], E25_W)
            res_t = r4(a6_t, F)

            # ---- PE warm-up (HAM clock gate): keep the PE busy from the
            # end of the framework preamble until real data lands ----
            warm_sb = wpool.tile([P, 512], f16, tag="warm")
            nc.vector.memset(warm_sb, 0.0)
            wps = mmps.tile([P, 512], f32, tag="mm", name="warm")
            for w in range(N_WARM):
                nc.tensor.matmul(
                    wps, warm_sb[:, 0:P], warm_sb,
                    start=(w == 0), stop=(w == N_WARM - 1),
                )

            ones_t = wpool.tile([P, P], f32, tag="ones")
            nc.vector.memset(ones_t, 1.0)

            # ---- persistent tiles ----
            qt_sb = proj.tile([P, 2, QB], f16, tag="qt")
            kt_b = [
                proj.tile([P, 2, 512], f16, tag=f"kt{nb}", name=f"kt{nb}")
                for nb in range(4)
            ]
            v_b = [
                proj.tile([P, 4, N_HEADS, 65], f16, tag=f"v{nb}", name=f"v{nb}")
                for nb in range(4)
            ]
            ctx_sb = proj.tile([P, 2, QB], f16, tag="ctx")
            ztmp_z = proj.tile([P, N_HEADS, QB], f32, tag="z")
            fcacc = proj.tile([P, 4, F], f32, tag="fcacc")
            o_sb = proj.tile([P, 4, F], f16, tag="osb")

            # ---- projections ----
            def project_qt():
                for g in range(2):
                    ps = mmps.tile([P, 512], f32, tag="mm", name=f"psq{g}")
                    for c in range(2):
                        nc.tensor.matmul(
                            ps,
                            wq_sb[:, c, g * P:(g + 1) * P],
                            xqt_sb[:, c, :],
                            start=(c == 0),
                            stop=(c == 1),
                        )
                    nc.vector.tensor_copy(qt_sb[:, g, :], ps)

            def project_kt(nb, groups=(0, 1)):
                for g in groups:
                    ps = mmps.tile([P, 512], f32, tag="mm", name=f"psk{nb}{g}")
                    for c in range(2):
                        nc.tensor.matmul(
                            ps,
                            wk_sb[:, c, g * P:(g + 1) * P],
                            xkt_b[nb][:, c, :],
                            start=(c == 0),
                            stop=(c == 1),
                        )
                    nc.vector.tensor_copy(kt_b[nb][:, g, :], ps)

            def project_v_pair(nb, prl):
                if prl == 0:
                    nc.vector.tensor_copy(
                        v_b[nb][:, :, :, 64:65],
                        ones_t[:, 0:4 * N_HEADS].rearrange(
                            "p (j h one) -> p j h one", j=4, h=N_HEADS, one=1,
                        ),
                    )
                ps = mmps.tile([P, 512], f32, tag="mm", name=f"psv{nb}{prl}")
                for j in range(2):
                    for c in range(2):
                        nc.tensor.matmul(
                            ps[:, j * F:(j + 1) * F],
                            xvt_b[nb][:, c, (2 * prl + j) * P:(2 * prl + j + 1) * P],
                            wv_sb[:, c, :],
                            start=(c == 0),
                            stop=(c == 1),
                        )
                nc.scalar.copy(
                    v_b[nb][:, 2 * prl:2 * prl + 2, :, 0:64],
                    ps.rearrange("p (j h d) -> p j h d", j=2, h=N_HEADS),
                )

            # ---- per-head attention chunk ----
            def band_cols(h, kc):
                if kc < 10:
                    return None
                t = kc - 10
                off_t = 256 - 128 * t
                j0 = max(0, -off_t)
                j1 = min(512, BAND[h] + 128 - off_t)
                j1 = min(512, (j1 + 7) & ~7)
                if j1 <= j0:
                    return None
                if t <= 1:
                    c0 = (128 - 128 * t) + j0
                    gsl = gs01_sb[:, h, c0:c0 + (j1 - j0)]
                    esl = e01_sb[:, h, c0:c0 + (j1 - j0)]
                else:
                    c0 = j0 - 128 * (t - 2)
                    gsl = gs25_sb[:, h, c0:c0 + (j1 - j0)]
                    esl = e25_sb[:, h, c0:c0 + (j1 - j0)]
                return j0, j1, gsl, esl

            def group_chunk(G, kc):
                """Score matmuls for both heads of a pair on one chunk,
                then ONE 1024-wide exp covering both heads (alternating
                engines per chunk). Band chunks get a fused exp(s+g)
                overwrite (DVE chunks) or an E-table multiply on the idle
                gpsimd engine (scalar chunks). Returns the fp16 pt tile
                laid out [128, head, 512]."""
                gg = G[0] // 2
                ps = spsum.tile([P, 2, QB], f32, tag="sc", name=f"sc{gg}_{kc}")
                for hi, h in enumerate(G):
                    po = (h % 2) * 64
                    nc.tensor.matmul(
                        ps[:, hi, :],
                        kt_b[kc // 4][po:po + 64, gg, (kc % 4) * P:(kc % 4 + 1) * P],
                        qt_sb[po:po + 64, gg, :],
                        start=True,
                        stop=True,
                    )
                pt = ptpool.tile([P, 2, QB], f16, tag="pt", name=f"pt{gg}_{kc}")
                if kc % 2 == 1:
                    nc.vector.tensor_scalar(
                        pt.bitcast(i16), ps, K_EXP, B_EXP,
                        op0=ALU.mult, op1=ALU.add,
                    )
                    for hi, h in enumerate(G):
                        bc = band_cols(h, kc)
                        if bc is not None:
                            j0, j1, gsl, _ = bc
                            nc.vector.scalar_tensor_tensor(
                                pt.bitcast(i16)[:, hi, j0:j1],
                                ps[:, hi, j0:j1], K_EXP, gsl,
                                op0=ALU.mult, op1=ALU.add,
                            )
                else:
                    nc.scalar.activation(pt, ps, AF.Exp)
                    for hi, h in enumerate(G):
                        bc = band_cols(h, kc)
                        if bc is not None:
                            j0, j1, _, esl = bc
                            nc.gpsimd.tensor_mul(
                                pt[:, hi, j0:j1], pt[:, hi, j0:j1], esl
                            )
                return pt

            def attn_pv(G, kc, pt):
                for hi, h in enumerate(G):
                    nc.tensor.matmul(
                        ctxps[h][0:65, :],
                        v_b[kc // 4][:, kc % 4, h, 0:65],
                        pt[:, hi, :],
                        start=(kc == 0),
                        stop=(kc == KC - 1),
                    )

            # ---- per-head 1/Z dance, emitted as drip-able closures ----
            def dance_steps(h, ctxps):
                gg, po = h // 2, (h % 2) * 64
                state = {}

                def d_copies():
                    nc.scalar.copy(ztmp_z[64:65, h, :], ctxps[64:65, :])
                    nc.vector.tensor_copy(
                        ctx_sb[po:po + 64, gg, :], ctxps[0:64, :]
                    )

                def d_fwd():
                    zt = mmps.tile([P, 512], f32, tag="mm", name=f"zt{h}")
                    for qc in range(4):
                        nc.tensor.transpose(
                            zt[:, qc:qc + 1],
                            ztmp_z[64:65, h, qc * P:(qc + 1) * P],
                            ident_f[64:65, 64:65],
                        )
                    ztc = opool.tile([P, 4], f32, tag="ztc", name=f"ztc{h}")
                    state["ztc"] = ztc
                    nc.vector.tensor_copy(ztc, zt[:, 0:4])
                    nc.vector.reciprocal(ztc, ztc)

                def d_back():
                    rz_ps = mmps.tile([P, 512], f32, tag="mm", name=f"rz{h}")
                    nc.tensor.transpose(rz_ps[0:4, 0:P], state["ztc"], ident_f)
                    rz4 = opool.tile([4, P], f16, tag="rz4", name=f"rz4{h}")
                    state["rz4"] = rz4
                    nc.vector.tensor_copy(rz4, rz_ps[0:4, 0:P])

                def d_zb():
                    zb = mmps.tile([P, 512], f32, tag="mm", name=f"zb{h}")
                    for qc in range(4):
                        nc.tensor.matmul(
                            zb[:, qc * P:(qc + 1) * P],
                            sel[0:4, qc, :],
                            state["rz4"],
                            start=True,
                            stop=True,
                        )
                    nc.vector.tensor_mul(
                        ctx_sb[po:po + 64, gg, :],
                        ctx_sb[po:po + 64, gg, :],
                        zb[po:po + 64, :],
                    )

                return [d_copies, d_fwd, d_back, d_zb]

            # ---- fc + residual (+ LayerNorm for gg=1) steps ----
            def fc_step(gg, qc):
                def emit():
                    pso = mmps.tile([P, 512], f32, tag="mm", name=f"pso{gg}{qc}")
                    pso = pso[:, :F]
                    nc.tensor.matmul(
                        pso,
                        ctx_sb[:, gg, qc * P:(qc + 1) * P],
                        wfc_sb[:, gg, :],
                        start=True,
                        stop=True,
                    )
                    if gg == 0:
                        nc.vector.tensor_add(fcacc[:, qc, :], fcacc[:, qc, :], pso)
                    else:
                        x_t = opool.tile([P, F], f32, tag="x", name=f"x{qc}")
                        nc.vector.tensor_add(x_t, pso, fcacc[:, qc, :])
                        st = opool.tile([P, 6], f32, tag="st", name=f"st{qc}")
                        nc.vector.bn_stats(st, x_t)
                        mv = opool.tile([P, 2], f32, tag="mv", name=f"mv{qc}")
                        nc.vector.bn_aggr(mv, st)
                        nc.scalar.activation(
                            mv[:, 1:2], mv[:, 1:2], AF.Sqrt,
                            bias=eps_t, scale=1.0,
                        )
                        nc.vector.reciprocal(mv[:, 1:2], mv[:, 1:2])
                        nc.vector.tensor_scalar(
                            o_sb[:, qc, :],
                            x_t,
                            mv[:, 0:1],
                            mv[:, 1:2],
                            op0=ALU.subtract,
                            op1=ALU.mult,
                        )
                        if qc == 1:
                            nc.sync.dma_start(out[:, 0:2, :], o_sb[:, 0:2, :])
                        elif qc == 3:
                            nc.sync.dma_start(out[:, 2:4, :], o_sb[:, 2:4, :])
                return emit

            # =============== group 0 (heads 0,1) + all projections =======
            NP = KC // 2  # 8 chunk-pairs
            G0, G1 = (0, 1), (2, 3)
            ctxps = [None] * N_HEADS
            for h in G0:
                ctxps[h] = cpsum.tile([P, QB], f32, tag="ctxp", name=f"ctxp{h}")
            project_qt()
            project_kt(0)

            # group-0 pass with projection drips one pair ahead of use
            ident_f = wpool.tile([P, P], f32, tag="identf")
            sel = wpool.tile([4, N_HEADS, P], f16, tag="sel")
            eps_t = wpool.tile([P, 1], f32, tag="eps")

            def make_consts():
                # epilogue constants, dripped into late group 0 where the
                # DVE has slack (no inter-group stall)
                make_identity(nc, ident_f)
                for hh in range(N_HEADS):
                    nc.vector.tensor_scalar_mul(
                        sel[0:4, hh, :], ones_t[0:4, :], ident_f[0:4, hh:hh + 1]
                    )
                nc.vector.memset(eps_t, LN_EPS)

            proj_drip = {
                0: [lambda: project_v_pair(0, 1)],
                1: [lambda: project_kt(1), lambda: project_v_pair(1, 0)],
                2: [lambda: project_v_pair(1, 1)],
                3: [lambda: project_kt(2), lambda: project_v_pair(2, 0)],
                4: [lambda: project_v_pair(2, 1)],
                5: [lambda: project_kt(3), lambda: project_v_pair(3, 0)],
                6: [lambda: project_v_pair(3, 1), make_consts],
            }
            pend = []
            for kc in range(KC):
                pt = group_chunk(G0, kc)
                if kc == 0:
                    project_v_pair(0, 0)
                if len(pend) >= 3:
                    pv = pend.pop(0)
                    attn_pv(G0, pv[0], pv[1])
                pend.append((kc, pt))
                if kc % 2 == 1:
                    for step in proj_drip.get(kc // 2, ()):
                        step()
            pend0 = pend

            # ====== group 1 pass with group-0 dances + fc(gg=0) dripped ===
            d0 = dance_steps(0, ctxps[0])
            d1 = dance_steps(1, ctxps[1])
            fc0 = [fc_step(0, qc) for qc in range(4)]

            def fcacc_init():
                nc.gpsimd.tensor_copy(fcacc, res_t)

            # start group 1's first chunks BEFORE draining group 0's last
            # PVs and emitting the ctx/z dance copies: the PE stays dense
            # across the handoff so the HAM clock gate never re-closes.
            pend = []
            for kc in range(3):
                pend.append((kc, group_chunk(G1, kc)))
            for pv in pend0:
                attn_pv(G0, pv[0], pv[1])
            d0[0]()
            d1[0]()
            for h in G1:
                ctxps[h] = cpsum.tile([P, QB], f32, tag="ctxp", name=f"ctxp{h}")
            drip = {
                1: [d0[1], fcacc_init], 2: [d0[2], d1[1]], 3: [d0[3], d1[2]],
                4: [d1[3]], 5: [fc0[0], fc0[1]],
                6: [fc0[2], fc0[3]],
            }
            for kc in range(3, KC):
                pt = group_chunk(G1, kc)
                if len(pend) >= 3:
                    pv = pend.pop(0)
                    attn_pv(G1, pv[0], pv[1])
                pend.append((kc, pt))
                if kc % 2 == 1:
                    for step in drip.get(kc // 2, ()):
                        step()
            for pv in pend:
                attn_pv(G1, pv[0], pv[1])

            # ---- tail: heads 2,3 dances interleaved + fc(gg=1) + LN ----
            d2 = dance_steps(2, ctxps[2])
            d3 = dance_steps(3, ctxps[3])
            for s2, s3 in zip(d2, d3):
                s2()
                s3()
            for qc in range(4):
                fc_step(1, qc)()

    nc.compile()
    return nc


def get_nc():
    if "nc" not in _CACHE:
        _CACHE["nc"] = _build_program()
    return _CACHE["nc"]


def make_in_maps(input_Q, input_K, input_V, W_Q, W_K, W_V, W_fc):
    c16 = lambda a: np.ascontiguousarray(
        np.asarray(a, dtype=np.float32), dtype=np.float16
    )
    # pack [in, out]-style matrices to SBUF layout [p, c, out]
    pk_w = lambda w: c16(np.asarray(w, np.float32).reshape(2, P, -1).transpose(1, 0, 2))
    # pack an activation block X [seq, F] to X^T SBUF layout [p, c, seq]
    pk_t = lambda x: c16(np.asarray(x, np.float32).T.reshape(2, P, -1).transpose(1, 0, 2))
    # pack a rolled key/value matrix [2048, F] to per-block X^T [nb, p, c, 512]
    pk_x = lambda x: c16(
        np.asarray(x, np.float32).reshape(4, 512, 2, P).transpose(0, 3, 2, 1)
    )
    gs01t, gs25t, e01t, e25t = _gauss_tables()
    gs01 = np.ascontiguousarray(gs01t.transpose(1, 0, 2))
    gs25 = np.ascontiguousarray(gs25t.transpose(1, 0, 2))
    e01 = np.ascontiguousarray(e01t.transpose(1, 0, 2))
    e25 = np.ascontiguousarray(e25t.transpose(1, 0, 2))
    gs01_neutral = np.full_like(gs01, np.float16(B_EXP))
    e01_neutral = np.ones_like(e01)
    wq8 = pk_w(np.asarray(W_Q, np.float32) / np.float32(np.sqrt(D_K)))
    wk = pk_w(W_K)
    wv = pk_w(W_V)
    wfc = pk_w(W_fc)
    flat = lambda a: np.asarray(a).reshape(P, -1)
    cat = lambda *parts: np.ascontiguousarray(
        np.concatenate([flat(p) for p in parts], axis=1)
    )
    in_maps = []
    for c in range(N_CORES):
        b, qb = divmod(c, 4)
        q0 = qb * QB
        # roll so the causal band (k in [q0-256, q0+512)) sits at the END
        # of the chunk sequence (slots 10-15): rolled[1280] = orig[q0-256]
        r = (q0 + 512) % S
        xq_blk = np.asarray(input_Q[b][q0:q0 + QB], np.float32)
        xk_rot = np.roll(np.asarray(input_K[b], np.float32), -r, axis=0)
        xv_rot = np.roll(np.asarray(input_V[b], np.float32), -r, axis=0)
        xkt = pk_x(xk_rot)
        xvt = pk_x(xv_rot)
        res = c16(xq_blk.reshape(4, P, F).transpose(1, 0, 2))
        in_maps.append({
            "s1": cat(pk_t(xq_blk), wq8),
            "s2": cat(wk, xkt[0]),
            "s3": cat(xkt[1]),
            "s4": cat(xkt[2]),
            "s5": cat(xkt[3]),
            "s6": cat(wfc),
            "a1": cat(wv, xvt[0]),
            "a2": cat(xvt[1]),
            "a3": cat(xvt[2]),
            "a4": cat(xvt[3]),
            "a5": cat(gs01_neutral if q0 == 0 else gs01, gs25,
                      e01_neutral if q0 == 0 else e01, e25),
            "a6": cat(res),
        })
    return in_maps


def kernel(input_Q, input_K, input_V, W_Q, W_K, W_V, W_fc, attn_mask=None):
    from concourse.bass_utils import run_bass_kernel_spmd

    nc = get_nc()
    in_maps = make_in_maps(input_Q, input_K, input_V, W_Q, W_K, W_V, W_fc)
    res = run_bass_kernel_spmd(nc, in_maps, core_ids=list(range(N_CORES)))
    out = np.empty((B, S, F), dtype=np.float32)
    for c in range(N_CORES):
        b, qb = divmod(c, 4)
        o = res.results[c]["out"]
        out[b, qb * QB:(qb + 1) * QB, :] = o.transpose(1, 0, 2).reshape(QB, F)
    return out
